# revision 1
# baseline (speedup 1.0000x reference)
"""Trainium2 Bass kernel for nn_CrossSpaceAttention (batch 8, DIM=128, HEADS=8,
128x128 spatial). Data-parallel over batch: one sample per NeuronCore x8.

Per-core algorithm (all derived host-side constants folded):
  q = sum_t diag(qdw_t) @ qw @ shift_t(x0) + bias(h,w)      (dense 3x3 conv, 9 matmuls/tile on PE)
  k = likewise from x1
  G[c,d] = sum_n q[c,n] k[d,n] per head (via PE transposes + PE Gram accumulation)
  attn = softmax(0.25 * G / (|q_c| |k_d|))  per 32x32 head block
  y = sum_s (pw @ blockdiag(attn) @ diag(vdw_s) vw) @ shift_s(x2) + bias'  (attn+proj folded into conv)

Biases (including SAME-padding border effects) are applied as per-partition
tensor_scalar adds at PSUM-evacuation time: interior constant + edge rows/cols
+ corner fixups (exact).
"""
import numpy as np
import ml_dtypes

import concourse.bass as bass
import concourse.bacc as bacc
import concourse.mybir as mybir
import concourse.tile as tile
from concourse.bass_utils import run_bass_kernel_spmd
from concourse.masks import make_identity

BF = mybir.dt.bfloat16
F32 = mybir.dt.float32
BF_NP = ml_dtypes.bfloat16

C = 128          # input channels (DIM)
D2 = 256         # qkv channels
HH = 128         # spatial H
WW = 128         # spatial W
PH, PW = HH + 2, WW + 2
NTILE = 32       # spatial tiles of 4 rows x 128 cols (512 elements)
TAPS = [(dy, dx) for dy in (-1, 0, 1) for dx in (-1, 0, 1)]
ADD = mybir.AluOpType.add
MULT = mybir.AluOpType.mult
AF = mybir.ActivationFunctionType

_CACHE = {}


def _conv_block(nc, j, acc, wts, xp, extra=None):
    """9 accumulated tap matmuls into psum tile acc for spatial tile j.

    wts: SBUF (128, 9, M) lhsT per tap; xp: padded input (128, PH, PW)."""
    for t, (dy, dx) in enumerate(TAPS):
        rhs = xp[:, 4 * j + 1 + dy:4 * j + 5 + dy, 1 + dx:1 + dx + WW]
        lhsT = wts[:, t, :] if extra is None else wts[:, t, extra[0]:extra[1]]
        nc.tensor.matmul(acc[:, :, :], lhsT, rhs, start=(t == 0), stop=(t == 8))


def _bias_fixups(nc, st, cols, m, j, last_row=3):
    """Edge/corner bias adds on an evacuated tile st (128, 4, 128).

    cols: (128, n_chunks, 9) bias columns {int,dt,db,dl,dr,tl,tr,bl,br};
    interior (col 0) is applied during evacuation, not here."""
    cs = lambda i: cols[:, m, i:i + 1]
    nc.vector.tensor_scalar(out=st[:, :, 0:1], in0=st[:, :, 0:1],
                            scalar1=cs(3), scalar2=None, op0=ADD)
    nc.vector.tensor_scalar(out=st[:, :, 127:128], in0=st[:, :, 127:128],
                            scalar1=cs(4), scalar2=None, op0=ADD)
    if j == 0:
        nc.vector.tensor_scalar(out=st[:, 0, :], in0=st[:, 0, :],
                                scalar1=cs(1), scalar2=None, op0=ADD)
        nc.vector.tensor_scalar(out=st[:, 0, 0:1], in0=st[:, 0, 0:1],
                                scalar1=cs(5), scalar2=None, op0=ADD)
        nc.vector.tensor_scalar(out=st[:, 0, 127:128], in0=st[:, 0, 127:128],
                                scalar1=cs(6), scalar2=None, op0=ADD)
    if j == NTILE - 1:
        nc.vector.tensor_scalar(out=st[:, last_row, :], in0=st[:, last_row, :],
                                scalar1=cs(2), scalar2=None, op0=ADD)
        nc.vector.tensor_scalar(out=st[:, last_row, 0:1], in0=st[:, last_row, 0:1],
                                scalar1=cs(7), scalar2=None, op0=ADD)
        nc.vector.tensor_scalar(out=st[:, last_row, 127:128], in0=st[:, last_row, 127:128],
                                scalar1=cs(8), scalar2=None, op0=ADD)


def _load_pad(nc, xp, xd):
    """Zero the pad border of xp (128, PH, PW) and DMA the image into the interior."""
    nc.vector.memset(xp[:, 0, :], 0.0)
    nc.vector.memset(xp[:, PH - 1, :], 0.0)
    nc.vector.memset(xp[:, 1:PH - 1, 0:1], 0.0)
    nc.vector.memset(xp[:, 1:PH - 1, PW - 1:PW], 0.0)
    nc.sync.dma_start(out=xp[:, 1:PH - 1, 1:PW - 1], in_=xd[:, :, :])


def _build_nc():
    nc = bacc.Bacc(None, target_bir_lowering=False)

    x0d = nc.dram_tensor("x0", (C, HH, WW), BF, kind="ExternalInput")
    x1d = nc.dram_tensor("x1", (C, HH, WW), BF, kind="ExternalInput")
    x2d = nc.dram_tensor("x2", (C, HH, WW), BF, kind="ExternalInput")
    aqd = nc.dram_tensor("aq", (C, 9, D2), BF, kind="ExternalInput")
    akd = nc.dram_tensor("ak", (C, 9, D2), BF, kind="ExternalInput")
    qcd = nc.dram_tensor("qcols", (C, 2, 9), F32, kind="ExternalInput")
    kcd = nc.dram_tensor("kcols", (C, 2, 9), F32, kind="ExternalInput")
    cvd = nc.dram_tensor("cv", (C, 9, 2, C), F32, kind="ExternalInput")
    pwtd = nc.dram_tensor("pwT", (C, 2, C), F32, kind="ExternalInput")
    bvd = nc.dram_tensor("bv", (C, 2, 9), F32, kind="ExternalInput")
    pbd = nc.dram_tensor("pbrow", (1, C), F32, kind="ExternalInput")
    e0d = nc.dram_tensor("e0row", (1, 9), F32, kind="ExternalInput")
    onesd = nc.dram_tensor("ones1", (1, C), F32, kind="ExternalInput")
    yd = nc.dram_tensor("y", (C, HH, WW), F32, kind="ExternalOutput")
    import os
    dbg = bool(os.environ.get("KDEBUG"))
    if dbg:
        gdumpd = nc.dram_tensor("gdump", (128, 2, 128), F32, kind="ExternalOutput")
        adumpd = nc.dram_tensor("adump", (128, 2, D2), F32, kind="ExternalOutput")
        ndumpd = nc.dram_tensor("ndump", (128, 4), F32, kind="ExternalOutput")
        edumpd = nc.dram_tensor("edump", (128, 9, C), BF, kind="ExternalOutput")
        cdumpd = nc.dram_tensor("cdump", (128, 9), F32, kind="ExternalOutput")
        qtdumpd = nc.dram_tensor("qtdump", (128, 128, D2), BF, kind="ExternalOutput")

    with tile.TileContext(nc) as tc:
        with (
            tc.tile_pool(name="consts", bufs=1) as consts,
            tc.tile_pool(name="xpad", bufs=2) as xpad,
            tc.tile_pool(name="qtp", bufs=1) as qtp,
            tc.tile_pool(name="ktile", bufs=6) as ktile,
            tc.tile_pool(name="stage", bufs=6) as stage,
            tc.tile_pool(name="sqscr", bufs=4) as sqscr,
            tc.tile_pool(name="small", bufs=1) as small,
            tc.tile_pool(name="ysb", bufs=6) as ysb,
            tc.tile_pool(name="cpsum", bufs=3, space="PSUM") as cpsum,
            tc.tile_pool(name="tpsum", bufs=2, space="PSUM") as tpsum,
            tc.tile_pool(name="gpsum", bufs=1, space="PSUM") as gpsum,
            tc.tile_pool(name="mpsum", bufs=1, space="PSUM") as mpsum,
        ):
            # ---- constants ----
            aq = consts.tile([C, 9, D2], BF)
            nc.sync.dma_start(out=aq, in_=aqd[:, :, :])
            ak = consts.tile([C, 9, D2], BF)
            nc.sync.dma_start(out=ak, in_=akd[:, :, :])
            qcols = consts.tile([C, 2, 9], F32)
            nc.sync.dma_start(out=qcols, in_=qcd[:, :, :])
            kcols = consts.tile([C, 2, 9], F32)
            nc.sync.dma_start(out=kcols, in_=kcd[:, :, :])
            cv = consts.tile([C, 9, 2, C], F32)
            nc.sync.dma_start(out=cv, in_=cvd[:, :, :, :])
            pwt = consts.tile([C, 2, C], F32)
            nc.sync.dma_start(out=pwt, in_=pwtd[:, :, :])
            bv = consts.tile([C, 2, 9], F32)
            nc.sync.dma_start(out=bv, in_=bvd[:, :, :])
            pbrow = consts.tile([1, C], F32)
            nc.sync.dma_start(out=pbrow, in_=pbd[:, :])
            e0row = consts.tile([1, 9], F32)
            nc.sync.dma_start(out=e0row, in_=e0d[:, :])
            ones1 = consts.tile([1, C], F32)
            nc.sync.dma_start(out=ones1, in_=onesd[:, :])
            identb = consts.tile([128, 128], BF)
            make_identity(nc, identb)
            identf = consts.tile([128, 128], F32)
            make_identity(nc, identf)

            # ---- accumulators / attn-stage tiles ----
            qT = qtp.tile([128, 128, D2], BF)     # [n_in_chunk, n_chunk, c]
            qn2 = small.tile([128, 2, NTILE], F32)
            kn2 = small.tile([128, 2, NTILE], F32)
            qinv = small.tile([128, 2], F32)
            kinv = small.tile([128, 2], F32)
            kir = small.tile([1, 2, C], F32)
            kb = small.tile([128, 2, C], F32)
            lblk = small.tile([128, 2, 32], F32)
            ablk = small.tile([128, 2, 32], F32)
            rs = small.tile([128, 2], F32)
            rr = small.tile([128, 2], F32)
            attnBD = small.tile([128, 2, D2], F32)
            pat = small.tile([128, 2, C], F32)
            eall = small.tile([128, 9, C], BF)
            coly = small.tile([128, 9], F32)

            x0p = xpad.tile([C, PH, PW], BF, tag="xp")
            _load_pad(nc, x0p, x0d)
            x1p = xpad.tile([C, PH, PW], BF, tag="xp")
            _load_pad(nc, x1p, x1d)

            nc.vector.memset(attnBD.rearrange("p a b -> p (a b)"), 0.0)

            # ---- q / k convs, staging, transposes, norms, gram ----
            for conv in ("q", "k"):
                wts, xp, cols, n2 = ((aq, x0p, qcols, qn2) if conv == "q"
                                     else (ak, x1p, kcols, kn2))
                for j in range(NTILE):
                    if conv == "k":
                        ktt = ktile.tile([128, 4, D2], BF)
                    for m in range(2):
                        acc = cpsum.tile([128, 4, 128], F32)
                        _conv_block(nc, j, acc, wts, xp, extra=(128 * m, 128 * m + 128))
                        st = stage.tile([128, 4, 128], BF)
                        nc.vector.tensor_scalar(out=st, in0=acc,
                                                scalar1=cols[:, m, 0:1],
                                                scalar2=None, op0=ADD)
                        _bias_fixups(nc, st, cols, m, j)
                        sq = sqscr.tile([128, 512], BF)
                        nc.scalar.activation(out=sq, in_=st.rearrange("p a b -> p (a b)"),
                                             func=AF.Square,
                                             accum_out=n2[:, m, j:j + 1])
                        tp = tpsum.tile([128, 4, 128], BF)
                        for b in range(4):
                            nc.tensor.transpose(tp[:, b, :], st[:, b, :], identb)
                        if conv == "q":
                            nc.scalar.copy(
                                qT[:, 4 * j:4 * j + 4, 128 * m:128 * m + 128],
                                tp[:, :, :])
                        else:
                            nc.scalar.copy(
                                ktt[:, :, 128 * m:128 * m + 128], tp[:, :, :])
                    if conv == "k":
                        if j == 0:
                            G0 = gpsum.tile([128, 128], F32, tag="G0")
                            G1 = gpsum.tile([128, 128], F32, tag="G1")
                        for b in range(4):
                            jn = 4 * j + b
                            for g, Gt in ((0, G0), (1, G1)):
                                nc.tensor.matmul(
                                    Gt[:, :],
                                    qT[:, jn, 128 * g:128 * g + 128],
                                    ktt[:, b, 128 * g:128 * g + 128],
                                    start=(jn == 0), stop=(jn == 4 * NTILE - 1))

            x2p = xpad.tile([C, PH, PW], BF, tag="xp")
            _load_pad(nc, x2p, x2d)

            # ---- norms -> qinv = 1/|q|, kinv = 0.25/|k| ----
            for m in range(2):
                nc.vector.tensor_reduce(out=qinv[:, m:m + 1], in_=qn2[:, m, :],
                                        axis=mybir.AxisListType.X, op=ADD)
                nc.vector.tensor_reduce(out=kinv[:, m:m + 1], in_=kn2[:, m, :],
                                        axis=mybir.AxisListType.X, op=ADD)
            nc.scalar.activation(out=qinv, in_=qinv, func=AF.Sqrt)
            nc.scalar.activation(out=kinv, in_=kinv, func=AF.Sqrt, scale=16.0)
            nc.vector.reciprocal(out=qinv, in_=qinv)
            nc.vector.reciprocal(out=kinv, in_=kinv)

            # broadcast kinv across partitions: kb[p, g, d] = kinv[d, g]
            for g in range(2):
                kt = mpsum.tile([1, C], F32, tag="mp")
                nc.tensor.transpose(kt, kinv[:, g:g + 1], identf)
                nc.vector.tensor_copy(kir[:, g, :], kt)
                kbp = mpsum.tile([128, C], F32, tag="mp")
                nc.tensor.matmul(kbp, ones1, kir[:, g, :], start=True, stop=True)
                nc.vector.tensor_copy(kb[:, g, :], kbp)

            # ---- softmax per 32x32 head block -> attnBD ----
            for g in range(2):
                for b in range(4):
                    p0 = 32 * b
                    Gt = G0 if g == 0 else G1
                    nc.vector.tensor_tensor(
                        out=lblk[p0:p0 + 32, g, :],
                        in0=Gt[p0:p0 + 32, p0:p0 + 32],
                        in1=kb[p0:p0 + 32, g, p0:p0 + 32],
                        op=MULT)
                    nc.scalar.activation(
                        out=ablk[p0:p0 + 32, g, :], in_=lblk[p0:p0 + 32, g, :],
                        func=AF.Exp, scale=qinv[p0:p0 + 32, g:g + 1],
                        accum_out=rs[p0:p0 + 32, g:g + 1])
                nc.vector.reciprocal(out=rr[:, g:g + 1], in_=rs[:, g:g + 1])
                for b in range(4):
                    p0 = 32 * b
                    nc.vector.tensor_scalar(
                        out=attnBD[p0:p0 + 32, g, 128 * g + p0:128 * g + p0 + 32],
                        in0=ablk[p0:p0 + 32, g, :],
                        scalar1=rr[p0:p0 + 32, g:g + 1], scalar2=None, op0=MULT)

            if dbg:
                gsb = small.tile([128, 2, 128], F32)
                nc.vector.tensor_copy(gsb[:, 0, :], G0)
                nc.vector.tensor_copy(gsb[:, 1, :], G1)
                nc.sync.dma_start(out=gdumpd[:, :, :], in_=gsb)
                nc.sync.dma_start(out=adumpd[:, :, :], in_=attnBD)
                nvd = small.tile([128, 4], F32)
                nc.vector.tensor_copy(nvd[:, 0:2], qinv)
                nc.vector.tensor_copy(nvd[:, 2:4], kinv)
                nc.sync.dma_start(out=ndumpd[:, :], in_=nvd)
                nc.sync.dma_start(out=qtdumpd[:, :, :], in_=qT)

            # ---- PA^T = attnBD^T @ pw^T ----
            patp = mpsum.tile([128, 2, C], F32, tag="mp")
            for mc in range(2):
                for kc in range(2):
                    nc.tensor.matmul(patp[:, mc, :],
                                     attnBD[:, kc, 128 * mc:128 * mc + 128],
                                     pwt[:, kc, :], start=(kc == 0), stop=(kc == 1))
            nc.vector.tensor_copy(pat.rearrange("p a b -> p (a b)"),
                                  patp.rearrange("p a b -> p (a b)"))

            # ---- E_s^T = C_s^T @ PA^T  (y-conv weights), and bias columns ----
            for s in range(9):
                ep = mpsum.tile([128, C], F32, tag="mp")
                for kc in range(2):
                    nc.tensor.matmul(ep, cv[:, s, kc, :], pat[:, kc, :],
                                     start=(kc == 0), stop=(kc == 1))
                nc.vector.tensor_copy(eall[:, s, :], ep)
            wp = mpsum.tile([128, 9], F32, tag="mp")
            for kc in range(2):
                nc.tensor.matmul(wp, pat[:, kc, :], bv[:, kc, :],
                                 start=(kc == 0), stop=False)
            nc.tensor.matmul(wp, pbrow, e0row, start=False, stop=True)
            nc.vector.tensor_copy(coly, wp)
            if dbg:
                nc.sync.dma_start(out=edumpd[:, :, :], in_=eall)
                nc.sync.dma_start(out=cdumpd[:, :], in_=coly)

            # ---- y conv ----
            for j in range(NTILE):
                acc = cpsum.tile([128, 4, 128], F32)
                _conv_block(nc, j, acc, eall, x2p)
                yt = ysb.tile([128, 4, 128], F32)
                nc.vector.tensor_scalar(out=yt, in0=acc, scalar1=coly[:, 0:1],
                                        scalar2=None, op0=ADD)
                _bias_fixups(nc, yt, coly.rearrange("p (a b) -> p a b", a=1), 0, j)
                nc.sync.dma_start(out=yd[:, 4 * j:4 * j + 4, :], in_=yt)

    nc.compile()
    return nc


def _host_consts(qw, qb, kw, kb, vw, vb, qdw, qdb, kdw, kdb, vdw, vdb, pw, pb):
    """Fold all static weights into the forms the kernel consumes."""
    qw2, kw2, vw2, pw2 = [w[:, :, 0, 0].astype(np.float64) for w in (qw, kw, vw, pw)]
    qd, kd, vd = [w[:, 0].astype(np.float64) for w in (qdw, kdw, vdw)]

    def conv_w(d, w2):
        # (C, 9, D2): lhsT per tap: A_t^T where A_t = diag(d_t) @ w2
        a = np.stack([(d[:, dy + 1, dx + 1][:, None] * w2).T
                      for (dy, dx) in TAPS], axis=1)
        return a.astype(np.float32).astype(BF_NP)

    def bias_cols(b1, db, d):
        cols = np.stack([
            db + b1 * d.sum((-2, -1)),
            -b1 * d[:, 0, :].sum(-1), -b1 * d[:, 2, :].sum(-1),
            -b1 * d[:, :, 0].sum(-1), -b1 * d[:, :, 2].sum(-1),
            b1 * d[:, 0, 0], b1 * d[:, 0, 2], b1 * d[:, 2, 0], b1 * d[:, 2, 2],
        ], axis=-1)  # (256, 9)
        return cols.reshape(2, 128, 9).transpose(1, 0, 2).astype(np.float32)

    cv = np.stack([(vd[:, dy + 1, dx + 1][:, None] * vw2)
                   for (dy, dx) in TAPS])             # (9, 256, 128)
    cv = cv.reshape(9, 2, 128, 128).transpose(2, 0, 1, 3).astype(np.float32)
    pwT = pw2.T.reshape(2, 128, 128).transpose(1, 0, 2).astype(np.float32)
    e0 = np.zeros((1, 9), np.float32)
    e0[0, 0] = 1.0
    return {
        "aq": conv_w(qd, qw2), "ak": conv_w(kd, kw2),
        "qcols": bias_cols(qb.astype(np.float64), qdb.astype(np.float64), qd),
        "kcols": bias_cols(kb.astype(np.float64), kdb.astype(np.float64), kd),
        "cv": cv, "pwT": pwT,
        "bv": bias_cols(vb.astype(np.float64), vdb.astype(np.float64), vd),
        "pbrow": pb.reshape(1, C).astype(np.float32),
        "e0row": e0,
        "ones1": np.ones((1, C), np.float32),
    }


def kernel(**inputs):
    if "nc" not in _CACHE:
        _CACHE["nc"] = _build_nc()
    nc = _CACHE["nc"]

    consts = _host_consts(**{k: np.asarray(inputs[k]) for k in
                             ("qw", "qb", "kw", "kb", "vw", "vb", "qdw", "qdb",
                              "kdw", "kdb", "vdw", "vdb", "pw", "pb")})
    x0 = np.asarray(inputs["x0"]).astype(BF_NP)
    x1 = np.asarray(inputs["x1"]).astype(BF_NP)
    x2 = np.asarray(inputs["x2"]).astype(BF_NP)
    n_cores = x0.shape[0]
    in_maps = [dict(consts, x0=x0[i], x1=x1[i], x2=x2[i]) for i in range(n_cores)]
    res = run_bass_kernel_spmd(nc, in_maps, list(range(n_cores)))
    _CACHE["last_res"] = res
    return np.stack([np.asarray(r["y"]) for r in res.results]).astype(np.float32)


def kernel_sim(**inputs):
    """CoreSim validation path: run sample 0 only through the simulator."""
    from concourse.bass_interp import CoreSim

    if "nc" not in _CACHE:
        _CACHE["nc"] = _build_nc()
    nc = _CACHE["nc"]
    consts = _host_consts(**{k: np.asarray(inputs[k]) for k in
                             ("qw", "qb", "kw", "kb", "vw", "vb", "qdw", "qdb",
                              "kdw", "kdb", "vdw", "vdb", "pw", "pb")})
    sim = CoreSim(nc)
    for name, arr in consts.items():
        sim.tensor(name)[:] = arr
    for name in ("x0", "x1", "x2"):
        sim.tensor(name)[:] = np.asarray(inputs[name])[0].astype(BF_NP)
    sim.simulate()
    return np.array(sim.tensor("y"))[None].astype(np.float32)



# revision 4
# speedup vs baseline: 1.9960x; 1.9960x over previous
"""Trainium2 Bass kernel for nn_CrossSpaceAttention (batch 8, DIM=128, HEADS=8,
128x128 spatial). Data-parallel over batch: one sample per NeuronCore x8.

v2: fp8e4 DoubleRow tensor-engine path for the q/k convolutions and Gram.

Per-core algorithm:
  qT[n,c] = sum_t x0shift_t(n,:)^T A_t^T   (fp8 DoubleRow: taps paired in the
            2x contraction dim; outputs land pixel-major = Gram-ready, no
            transposes)
  kT likewise; per 128-pixel row chunk, PSUM -> fp8 SBUF qkT8 (pure copy).
  Per row-pair chunk and head-group g: DoubleRow Gram matmuls accumulate
    [q_g^T q_g | q_g^T k_g]  (self block gives norms on its diagonal),
    k_g^T k_g, and ones-column sums Sq, Sk.
  Bias (incl. its uniform interior part; SAME-border deltas are negligible
  for the attention path - verified numerically) is applied algebraically:
  rank-2 f32 correction matmuls  G += bq (x) (Sk + N bk) + Sq (x) bk  close
  each accumulation group exactly.
  attn = softmax(0.25 * G / (|q| |k|)) per 32x32 head block  (norms from the
  corrected self-Gram diagonals).
  y = sum_s (pw @ blockdiag(attn) @ diag(vdw_s) vw) @ shift_s(x2) + bias'
  (attn+proj folded into a bf16 3x3 conv, exact border bias columns).
"""
import numpy as np
import ml_dtypes

import concourse.bass as bass
import concourse.bacc as bacc
import concourse.mybir as mybir
import concourse.tile as tile
from concourse.bass_utils import run_bass_kernel_spmd
from concourse.masks import make_identity

BF = mybir.dt.bfloat16
F32 = mybir.dt.float32
FP8 = mybir.dt.float8e4
BF_NP = ml_dtypes.bfloat16
E4_NP = ml_dtypes.float8_e4m3
DR = mybir.MatmulPerfMode.DoubleRow

C = 128          # input channels (DIM)
D2 = 256         # qkv channels
HH = 128         # spatial H
WW = 128         # spatial W
PH, PW = HH + 2, WW + 2
PITCH = 144      # fp8 plane row pitch (16B-aligned for DoubleRow pair strides)
NPIX = HH * WW
SW = 8.0         # fp8 weight scale
SX = 16.0        # fp8 input scale  (q~tilde units = SW*SX*q = 128 q)
NTILE = 32       # y-conv spatial tiles of 4 rows x 128 cols
TAPS = [(dy, dx) for dy in (-1, 0, 1) for dx in (-1, 0, 1)]
# DoubleRow tap pairs: 3 vertical (dy=-1 with dy=0, delta=PITCH) and 3
# self-pairs (dy=+1 taps twice with halved weights, delta=0)
PAIRS = [(0, 3), (1, 4), (2, 5), (6, 6), (7, 7), (8, 8)]
ADD = mybir.AluOpType.add
MULT = mybir.AluOpType.mult
AF = mybir.ActivationFunctionType

_CACHE = {}


def _win_pair_ap(xp, y, p):
    """lhsT AP for conv row y, DoubleRow pair p: (128 cin, 2, 128 px)."""
    t0, t1 = PAIRS[p]
    dy0, dx0 = TAPS[t0]
    dy1, dx1 = TAPS[t1]
    off0 = (1 + y + dy0) * PITCH + 1 + dx0
    off1 = (1 + y + dy1) * PITCH + 1 + dx1
    return bass.AP(xp.tensor, xp.offset + off0,
                   [list(xp.ap[0]), [off1 - off0, 2], [1, 128]])


def _qk_ap(qkT8, pc, col, ncols):
    """(128, 2, ncols) pair AP over rows (2pc, 2pc+1) of qkT8 at col offset."""
    return bass.AP(qkT8.tensor, qkT8.offset + 2 * pc * 512 + col,
                   [list(qkT8.ap[0]), [512, 2], [1, ncols]])


def _evac_out_ap(qkT8, j, col):
    """(128, 2, 2, 128) write AP: rows (2j, 2j+1) x col blocks {col, col+256}."""
    return bass.AP(qkT8.tensor, qkT8.offset + 2 * j * 512 + col,
                   [list(qkT8.ap[0]), [512, 2], [256, 2], [1, 128]])


def _conv_block(nc, j, acc, wts, xp):
    """9 accumulated bf16 tap matmuls into psum tile acc for y-conv tile j."""
    for t, (dy, dx) in enumerate(TAPS):
        rhs = xp[:, 4 * j + 1 + dy:4 * j + 5 + dy, 1 + dx:1 + dx + WW]
        nc.tensor.matmul(acc[:, :, :], wts[:, t, :], rhs, start=(t == 0),
                         stop=(t == 8))


def _bias_fixups(nc, st, cols, j):
    """Edge/corner bias adds on an evacuated y tile st (128, 4, 128)."""
    cs = lambda i: cols[:, i:i + 1]
    nc.vector.tensor_scalar(out=st[:, :, 0:1], in0=st[:, :, 0:1],
                            scalar1=cs(3), scalar2=None, op0=ADD)
    nc.vector.tensor_scalar(out=st[:, :, 127:128], in0=st[:, :, 127:128],
                            scalar1=cs(4), scalar2=None, op0=ADD)
    if j == 0:
        nc.vector.tensor_scalar(out=st[:, 0, :], in0=st[:, 0, :],
                                scalar1=cs(1), scalar2=None, op0=ADD)
        nc.vector.tensor_scalar(out=st[:, 0, 0:1], in0=st[:, 0, 0:1],
                                scalar1=cs(5), scalar2=None, op0=ADD)
        nc.vector.tensor_scalar(out=st[:, 0, 127:128], in0=st[:, 0, 127:128],
                                scalar1=cs(6), scalar2=None, op0=ADD)
    if j == NTILE - 1:
        nc.vector.tensor_scalar(out=st[:, 3, :], in0=st[:, 3, :],
                                scalar1=cs(2), scalar2=None, op0=ADD)
        nc.vector.tensor_scalar(out=st[:, 3, 0:1], in0=st[:, 3, 0:1],
                                scalar1=cs(7), scalar2=None, op0=ADD)
        nc.vector.tensor_scalar(out=st[:, 3, 127:128], in0=st[:, 3, 127:128],
                                scalar1=cs(8), scalar2=None, op0=ADD)


def _load_pad(nc, xp, xd, pw):
    """Zero the pad border of xp (128, PH, pw) and DMA the image interior.

    Only columns 0..130 are ever read by conv windows; zero cols 0, 129, 130."""
    nc.vector.memset(xp[:, 0, 0:131], 0.0)
    nc.vector.memset(xp[:, PH - 1, 0:131], 0.0)
    nc.vector.memset(xp[:, 1:PH - 1, 0:1], 0.0)
    nc.vector.memset(xp[:, 1:PH - 1, 129:131], 0.0)
    nc.sync.dma_start(out=xp[:, 1:PH - 1, 1:129], in_=xd[:, :, :])


def _build_nc():
    nc = bacc.Bacc(None, target_bir_lowering=False)

    x0d = nc.dram_tensor("x0", (C, HH, WW), FP8, kind="ExternalInput")
    x1d = nc.dram_tensor("x1", (C, HH, WW), FP8, kind="ExternalInput")
    x2d = nc.dram_tensor("x2", (C, HH, WW), BF, kind="ExternalInput")
    aqd = nc.dram_tensor("aq8", (C, 12, D2), FP8, kind="ExternalInput")
    akd = nc.dram_tensor("ak8", (C, 12, D2), FP8, kind="ExternalInput")
    bcd = nc.dram_tensor("bcol", (C, 8), F32, kind="ExternalInput")
    cvd = nc.dram_tensor("cv", (C, 9, 2, C), F32, kind="ExternalInput")
    pwtd = nc.dram_tensor("pwT", (C, 2, C), F32, kind="ExternalInput")
    bvd = nc.dram_tensor("bv", (C, 2, 9), F32, kind="ExternalInput")
    pbd = nc.dram_tensor("pbrow", (1, C), F32, kind="ExternalInput")
    e0d = nc.dram_tensor("e0row", (1, 9), F32, kind="ExternalInput")
    onesd = nc.dram_tensor("ones1", (1, C), F32, kind="ExternalInput")
    yd = nc.dram_tensor("y", (C, HH, WW), F32, kind="ExternalOutput")
    import os
    dbg = bool(os.environ.get("KDEBUG"))
    if dbg:
        gdumpd = nc.dram_tensor("gdump", (128, 2, 256), F32, kind="ExternalOutput")
        sdumpd = nc.dram_tensor("sdump", (128, 2, 128), F32, kind="ExternalOutput")
        ndumpd = nc.dram_tensor("ndump", (128, 8), F32, kind="ExternalOutput")
        adumpd = nc.dram_tensor("adump", (128, 2, D2), F32, kind="ExternalOutput")
        edumpd = nc.dram_tensor("edump", (128, 9, C), BF, kind="ExternalOutput")
        qdumpd = nc.dram_tensor("qdump", (128, 4, 512), FP8, kind="ExternalOutput")

    with tile.TileContext(nc) as tc:
        with (
            tc.tile_pool(name="consts", bufs=1) as consts,
            tc.tile_pool(name="xpad8", bufs=2) as xpad8,
            tc.tile_pool(name="xpad16", bufs=1) as xpad16,
            tc.tile_pool(name="qkp", bufs=1) as qkp,
            tc.tile_pool(name="small", bufs=1) as small,
            tc.tile_pool(name="ysb", bufs=4) as ysb,
            tc.tile_pool(name="cpsum", bufs=3, space="PSUM") as cpsum,
            tc.tile_pool(name="gpsum", bufs=1, space="PSUM") as gpsum,
            tc.tile_pool(name="ypsum", bufs=2, space="PSUM") as ypsum,
            tc.tile_pool(name="mpsum", bufs=1, space="PSUM") as mpsum,
        ):
            # ---- constants ----
            aq8 = consts.tile([C, 12, D2], FP8)
            nc.sync.dma_start(out=aq8, in_=aqd[:, :, :])
            ak8 = consts.tile([C, 12, D2], FP8)
            nc.sync.dma_start(out=ak8, in_=akd[:, :, :])
            bcol = consts.tile([C, 8], F32)
            nc.sync.dma_start(out=bcol, in_=bcd[:, :])
            cv = consts.tile([C, 9, 2, C], F32)
            nc.sync.dma_start(out=cv, in_=cvd[:, :, :, :])
            pwt = consts.tile([C, 2, C], F32)
            nc.sync.dma_start(out=pwt, in_=pwtd[:, :, :])
            bv = consts.tile([C, 2, 9], F32)
            nc.sync.dma_start(out=bv, in_=bvd[:, :, :])
            pbrow = consts.tile([1, C], F32)
            nc.sync.dma_start(out=pbrow, in_=pbd[:, :])
            e0row = consts.tile([1, 9], F32)
            nc.sync.dma_start(out=e0row, in_=e0d[:, :])
            ones1 = consts.tile([1, C], F32)
            nc.sync.dma_start(out=ones1, in_=onesd[:, :])
            identf = consts.tile([128, 128], F32)
            make_identity(nc, identf)
            ones8t = consts.tile([128, 2, 16], FP8)
            nc.vector.memset(ones8t.rearrange("p a b -> p (a b)"), 1.0)
            ones8 = ones8t[:, :, 0:1]

            # ---- big SBUF tensors ----
            qkT8 = qkp.tile([128, 128, 512], FP8)   # [px, row, q0|k0|q1|k1]

            x0p = xpad8.tile([C, PH, PITCH], FP8, tag="xp8")
            _load_pad(nc, x0p, x0d, PITCH)
            x1p = xpad8.tile([C, PH, PITCH], FP8, tag="xp8")
            _load_pad(nc, x1p, x1d, PITCH)
            x2p = xpad16.tile([C, PH, PW], BF, tag="xp16")
            nc.vector.memset(x2p[:, 0, :], 0.0)
            nc.vector.memset(x2p[:, PH - 1, :], 0.0)
            nc.vector.memset(x2p[:, 1:PH - 1, 0:1], 0.0)
            nc.vector.memset(x2p[:, 1:PH - 1, PW - 1:PW], 0.0)
            nc.sync.dma_start(out=x2p[:, 1:PH - 1, 1:PW - 1], in_=x2d[:, :, :])

            # ---- small tiles ----
            scr = small.tile([128, 2, 8], F32)        # pre-transpose columns (g)
            stk = small.tile([2, 2, 4, 128], F32)     # [row, g, which, c] stacks
            dscr = small.tile([128, 128], F32)
            dq2 = small.tile([128, 2], F32)
            dk2 = small.tile([128, 2], F32)
            qinv = small.tile([128, 2], F32)
            kinv = small.tile([128, 2], F32)
            kir = small.tile([1, 2, C], F32)
            kb = small.tile([128, 2, C], F32)
            lblk = small.tile([128, 2, 32], F32)
            ablk = small.tile([128, 2, 32], F32)
            rs = small.tile([128, 2], F32)
            rr = small.tile([128, 2], F32)
            attnBD = small.tile([128, 2, D2], F32)
            pat = small.tile([128, 2, C], F32)
            eall = small.tile([128, 9, C], BF)
            coly = small.tile([128, 9], F32)

            evac_fns = [lambda o, i: nc.vector.tensor_copy(o, i),
                        lambda o, i: nc.scalar.copy(o, i)]
            Gt = gpsum.tile([128, 2, D2], F32, tag="G")
            sks = gpsum.tile([128, 260], F32, tag="sk")

            # ---- q conv (fp8 DoubleRow, qT-direct layout) ----
            for conv, (wts, xp, colbase) in enumerate(
                    ((aq8, x0p, 0), (ak8, x1p, 128))):
                for j in range(64):
                    acc = cpsum.tile([128, 512], F32, tag="cacc")
                    for r in range(2):
                        y = 2 * j + r
                        o = acc[:, 256 * r:256 * r + 256]
                        for p in range(6):
                            nc.tensor.matmul(o, _win_pair_ap(xp, y, p),
                                             wts[:, 2 * p:2 * p + 2, :],
                                             start=(p == 0), stop=(p == 5),
                                             perf_mode=DR)
                    src = acc.rearrange("p (r b i) -> p r b i", r=2, b=2)
                    evac_fns[j % 2](_evac_out_ap(qkT8, j, colbase), src)

                    # gram for pair-chunk pc (lagged) during the k conv
                    if conv == 1:
                        pcs = [j - 2] if j >= 2 else []
                        if j == 63:
                            pcs = [61, 62, 63]
                        for pc in pcs:
                            for g in range(2):
                                qg = _qk_ap(qkT8, pc, 256 * g, 128)
                                kg = _qk_ap(qkT8, pc, 256 * g + 128, 128)
                                nc.tensor.matmul(
                                    Gt[:, g, :], qg, _qk_ap(qkT8, pc, 256 * g, 256),
                                    start=(pc == 0), stop=False, perf_mode=DR,
                                    skip_group_check=True)
                                nc.tensor.matmul(
                                    sks[:, 128 * g:128 * g + 128], kg, kg,
                                    start=(pc == 0), stop=False, perf_mode=DR,
                                    skip_group_check=True)
                                nc.tensor.matmul(
                                    sks[:, 256 + 2 * g:257 + 2 * g], qg, ones8,
                                    start=(pc == 0), stop=(pc == 63),
                                    perf_mode=DR, skip_group_check=True)
                                nc.tensor.matmul(
                                    sks[:, 257 + 2 * g:258 + 2 * g], kg, ones8,
                                    start=(pc == 0), stop=(pc == 63),
                                    perf_mode=DR, skip_group_check=True)

            # ---- bias corrections: column scratch -> transposed stacks ----
            # scr cols (per g): [bq, Sq, bk, Sk, Sq+Nbq, bq, Sk+Nbk, bk]
            for g in range(2):
                sq_c = sks[:, 256 + 2 * g:257 + 2 * g]
                sk_c = sks[:, 257 + 2 * g:258 + 2 * g]
                nc.vector.tensor_copy(scr[:, g, 0:1], bcol[:, g:g + 1])
                nc.vector.tensor_copy(scr[:, g, 1:2], sq_c)
                nc.vector.tensor_copy(scr[:, g, 2:3], bcol[:, 2 + g:3 + g])
                nc.vector.tensor_copy(scr[:, g, 3:4], sk_c)
                nc.vector.tensor_tensor(out=scr[:, g, 4:5], in0=sq_c,
                                        in1=bcol[:, 4 + g:5 + g], op=ADD)
                nc.vector.tensor_copy(scr[:, g, 5:6], bcol[:, g:g + 1])
                nc.vector.tensor_tensor(out=scr[:, g, 6:7], in0=sk_c,
                                        in1=bcol[:, 6 + g:7 + g], op=ADD)
                nc.vector.tensor_copy(scr[:, g, 7:8], bcol[:, 2 + g:3 + g])
                for w in range(4):
                    tp2 = mpsum.tile([2, 128], F32, tag="mp")
                    nc.tensor.transpose(tp2, scr[:, g, 2 * w:2 * w + 2], identf)
                    nc.vector.tensor_copy(stk[:, g, w, :], tp2)
            # rank-2 closes: stacks w: 0=[bq;Sq], 1=[bk;Sk], 2=[Sq+Nbq;bq],
            # 3=[Sk+Nbk;bk]
            for g in range(2):
                nc.tensor.matmul(Gt[:, g, 0:128], stk[:, g, 0, :],
                                 stk[:, g, 2, :], start=False, stop=True,
                                 skip_group_check=True)
                nc.tensor.matmul(Gt[:, g, 128:256], stk[:, g, 0, :],
                                 stk[:, g, 3, :], start=False, stop=True,
                                 skip_group_check=True)
                nc.tensor.matmul(sks[:, 128 * g:128 * g + 128],
                                 stk[:, g, 1, :], stk[:, g, 3, :],
                                 start=False, stop=True,
                                 skip_group_check=True)

            # ---- norms: diag of corrected self-gram blocks -> rsqrt ----
            for g in range(2):
                nc.vector.tensor_tensor(out=dscr, in0=Gt[:, g, 0:128],
                                        in1=identf, op=MULT)
                nc.vector.tensor_reduce(out=dq2[:, g:g + 1], in_=dscr,
                                        axis=mybir.AxisListType.X, op=ADD)
                nc.vector.tensor_tensor(out=dscr, in0=sks[:, 128 * g:128 * g + 128],
                                        in1=identf, op=MULT)
                nc.vector.tensor_reduce(out=dk2[:, g:g + 1], in_=dscr,
                                        axis=mybir.AxisListType.X, op=ADD)
            nc.scalar.activation(out=qinv, in_=dq2, func=AF.Sqrt)
            nc.scalar.activation(out=kinv, in_=dk2, func=AF.Sqrt, scale=16.0)
            nc.vector.reciprocal(out=qinv, in_=qinv)
            nc.vector.reciprocal(out=kinv, in_=kinv)

            # ---- softmax per 32x32 head block -> attnBD ----
            nc.vector.memset(attnBD.rearrange("p a b -> p (a b)"), 0.0)
            for g in range(2):
                kt = mpsum.tile([1, C], F32, tag="mp")
                nc.tensor.transpose(kt, kinv[:, g:g + 1], identf)
                nc.vector.tensor_copy(kir[:, g, :], kt)
                kbp = mpsum.tile([128, C], F32, tag="mp")
                nc.tensor.matmul(kbp, ones1, kir[:, g, :], start=True, stop=True)
                nc.vector.tensor_copy(kb[:, g, :], kbp)
            for g in range(2):
                for b in range(4):
                    p0 = 32 * b
                    nc.vector.tensor_tensor(
                        out=lblk[p0:p0 + 32, g, :],
                        in0=Gt[p0:p0 + 32, g, 128 + p0:128 + p0 + 32],
                        in1=kb[p0:p0 + 32, g, p0:p0 + 32],
                        op=MULT)
                    nc.scalar.activation(
                        out=ablk[p0:p0 + 32, g, :], in_=lblk[p0:p0 + 32, g, :],
                        func=AF.Exp, scale=qinv[p0:p0 + 32, g:g + 1],
                        accum_out=rs[p0:p0 + 32, g:g + 1])
                nc.vector.reciprocal(out=rr[:, g:g + 1], in_=rs[:, g:g + 1])
                for b in range(4):
                    p0 = 32 * b
                    nc.vector.tensor_scalar(
                        out=attnBD[p0:p0 + 32, g, 128 * g + p0:128 * g + p0 + 32],
                        in0=ablk[p0:p0 + 32, g, :],
                        scalar1=rr[p0:p0 + 32, g:g + 1], scalar2=None, op0=MULT)

            if dbg:
                gsb = small.tile([128, 2, 256], F32)
                ssd = small.tile([128, 2, 128], F32)
                for g in range(2):
                    nc.vector.tensor_copy(gsb[:, g, :], Gt[:, g, :])
                    nc.vector.tensor_copy(ssd[:, g, :],
                                          sks[:, 128 * g:128 * g + 128])
                nc.sync.dma_start(out=gdumpd[:, :, :], in_=gsb)
                nc.sync.dma_start(out=sdumpd[:, :, :], in_=ssd)
                nvd = small.tile([128, 8], F32)
                nc.vector.tensor_copy(nvd[:, 0:2], qinv)
                nc.vector.tensor_copy(nvd[:, 2:4], kinv)
                nc.vector.tensor_copy(nvd[:, 4:6], dq2)
                nc.vector.tensor_copy(nvd[:, 6:8], dk2)
                nc.sync.dma_start(out=ndumpd[:, :], in_=nvd)
                nc.sync.dma_start(out=adumpd[:, :, :], in_=attnBD)
                nc.sync.dma_start(out=qdumpd[:, :, :], in_=qkT8[:, 0:4, :])

            # ---- PA^T = attnBD^T @ pw^T ----
            patp = mpsum.tile([128, 2, C], F32, tag="mp")
            for mc in range(2):
                for kc in range(2):
                    nc.tensor.matmul(patp[:, mc, :],
                                     attnBD[:, kc, 128 * mc:128 * mc + 128],
                                     pwt[:, kc, :], start=(kc == 0), stop=(kc == 1))
            nc.vector.tensor_copy(pat.rearrange("p a b -> p (a b)"),
                                  patp.rearrange("p a b -> p (a b)"))

            # ---- E_s^T = C_s^T @ PA^T  (y-conv weights), and bias columns ----
            for s in range(9):
                ep = mpsum.tile([128, C], F32, tag="mp")
                for kc in range(2):
                    nc.tensor.matmul(ep, cv[:, s, kc, :], pat[:, kc, :],
                                     start=(kc == 0), stop=(kc == 1))
                nc.vector.tensor_copy(eall[:, s, :], ep)
            wp = mpsum.tile([128, 9], F32, tag="mp")
            for kc in range(2):
                nc.tensor.matmul(wp, pat[:, kc, :], bv[:, kc, :],
                                 start=(kc == 0), stop=False)
            nc.tensor.matmul(wp, pbrow, e0row, start=False, stop=True)
            nc.vector.tensor_copy(coly, wp)
            if dbg:
                nc.sync.dma_start(out=edumpd[:, :, :], in_=eall)

            # ---- y conv (bf16) ----
            for j in range(NTILE):
                acc = ypsum.tile([128, 4, 128], F32, tag="yacc")
                _conv_block(nc, j, acc, eall, x2p)
                yt = ysb.tile([128, 4, 128], F32, tag="yt")
                nc.scalar.activation(out=yt.rearrange("p a b -> p (a b)"),
                                     in_=acc.rearrange("p a b -> p (a b)"),
                                     func=AF.Identity, bias=coly[:, 0:1])
                _bias_fixups(nc, yt, coly, j)
                nc.sync.dma_start(out=yd[:, 4 * j:4 * j + 4, :], in_=yt)

    nc.compile()
    return nc


def _host_consts(qw, qb, kw, kb, vw, vb, qdw, qdb, kdw, kdb, vdw, vdb, pw, pb):
    """Fold all static weights into the forms the kernel consumes."""
    qw2, kw2, vw2, pw2 = [w[:, :, 0, 0].astype(np.float64) for w in (qw, kw, vw, pw)]
    qd, kd, vd = [w[:, 0].astype(np.float64) for w in (qdw, kdw, vdw)]

    def conv_w8(d, w2):
        # (C, 12, D2) fp8 rhs, slots (2p, 2p+1) = the DoubleRow pair p
        a = np.zeros((C, 12, D2), np.float32)
        for p, (t0, t1) in enumerate(PAIRS):
            h = 0.5 if t0 == t1 else 1.0
            for s_, t in ((0, t0), (1, t1)):
                dy, dx = TAPS[t]
                a[:, 2 * p + s_, :] = (h * SW * d[:, dy + 1, dx + 1][:, None] * w2).T
        return a.astype(E4_NP)

    def bias_cols(b1, db, d):
        cols = np.stack([
            db + b1 * d.sum((-2, -1)),
            -b1 * d[:, 0, :].sum(-1), -b1 * d[:, 2, :].sum(-1),
            -b1 * d[:, :, 0].sum(-1), -b1 * d[:, :, 2].sum(-1),
            b1 * d[:, 0, 0], b1 * d[:, 0, 2], b1 * d[:, 2, 0], b1 * d[:, 2, 2],
        ], axis=-1)  # (256, 9)
        return cols.reshape(2, 128, 9).transpose(1, 0, 2).astype(np.float32)

    # interior bias in q~tilde units (x SW*SX)
    bq = (SW * SX) * (qdb.astype(np.float64) + qb.astype(np.float64) * qd.sum((-2, -1)))
    bk = (SW * SX) * (kdb.astype(np.float64) + kb.astype(np.float64) * kd.sum((-2, -1)))
    bcol = np.zeros((C, 8), np.float64)
    for g in range(2):
        bcol[:, g] = bq[128 * g:128 * g + 128]
        bcol[:, 2 + g] = bk[128 * g:128 * g + 128]
        bcol[:, 4 + g] = NPIX * bcol[:, g]
        bcol[:, 6 + g] = NPIX * bcol[:, 2 + g]

    cvf = np.stack([(vd[:, dy + 1, dx + 1][:, None] * vw2)
                    for (dy, dx) in TAPS])             # (9, 256, 128)
    cvf = cvf.reshape(9, 2, 128, 128).transpose(2, 0, 1, 3).astype(np.float32)
    pwT = pw2.T.reshape(2, 128, 128).transpose(1, 0, 2).astype(np.float32)
    e0 = np.zeros((1, 9), np.float32)
    e0[0, 0] = 1.0
    return {
        "aq8": conv_w8(qd, qw2), "ak8": conv_w8(kd, kw2),
        "bcol": bcol.astype(np.float32),
        "cv": cvf, "pwT": pwT,
        "bv": bias_cols(vb.astype(np.float64), vdb.astype(np.float64), vd),
        "pbrow": pb.reshape(1, C).astype(np.float32),
        "e0row": e0,
        "ones1": np.ones((1, C), np.float32),
    }


def _host_inputs(inputs):
    consts = _host_consts(**{k: np.asarray(inputs[k]) for k in
                             ("qw", "qb", "kw", "kb", "vw", "vb", "qdw", "qdb",
                              "kdw", "kdb", "vdw", "vdb", "pw", "pb")})
    x0 = (np.asarray(inputs["x0"]).astype(np.float32) * SX).astype(E4_NP)
    x1 = (np.asarray(inputs["x1"]).astype(np.float32) * SX).astype(E4_NP)
    x2 = np.asarray(inputs["x2"]).astype(BF_NP)
    return consts, x0, x1, x2


def kernel(**inputs):
    if "nc" not in _CACHE:
        _CACHE["nc"] = _build_nc()
    nc = _CACHE["nc"]
    consts, x0, x1, x2 = _host_inputs(inputs)
    n_cores = x0.shape[0]
    in_maps = [dict(consts, x0=x0[i], x1=x1[i], x2=x2[i]) for i in range(n_cores)]
    res = run_bass_kernel_spmd(nc, in_maps, list(range(n_cores)))
    _CACHE["last_res"] = res
    return np.stack([np.asarray(r["y"]) for r in res.results]).astype(np.float32)


def kernel_sim(**inputs):
    """CoreSim validation path: run sample 0 only through the simulator."""
    from concourse.bass_interp import CoreSim

    if "nc" not in _CACHE:
        _CACHE["nc"] = _build_nc()
    nc = _CACHE["nc"]
    consts, x0, x1, x2 = _host_inputs(inputs)
    sim = CoreSim(nc)
    for name, arr in consts.items():
        sim.tensor(name)[:] = arr
    sim.tensor("x0")[:] = x0[0]
    sim.tensor("x1")[:] = x1[0]
    sim.tensor("x2")[:] = x2[0]
    sim.simulate()
    _CACHE["sim"] = sim
    return np.array(sim.tensor("y"))[None].astype(np.float32)
